# revision 45
# baseline (speedup 1.0000x reference)
"""Trainium2 Bass kernel for a custom LSTM cell with LayerNorms.

Data-parallel across 8 NeuronCores: batch B=8192 is split into 8 shards of
1024 rows; weights are replicated.

Dataflow (v2):
  - comb = tanh(LN([x W_proj^T ; h])) is built feature-major ([feature,
    batch] tiles) exactly once: x/h/W_proj are transposed on the PE, the
    concat-LN statistics are ones-vector matmuls accumulated in one PSUM
    bank, and the mean/rstd rows are broadcast via a DRAM roundtrip.
  - The four gate matmuls produce BATCH-major outputs: the stationary
    operand is a [128k, 128b] slice of comb, the moving operand is a
    [128k, 512f] slice of W^T obtained by XBAR DMA-transpose from a bf16
    copy of W (written once by a fp32->bf16 cast-during-DMA on the SWDGE
    path, chunked and emitted one gate ahead so casts overlap matmuls).
    k is the outer loop so all 8 batch-chunk PSUM banks accumulate in
    parallel and only a handful of W^T tiles are resident.
  - Batch-major layout makes every per-batch LayerNorm a free-dim problem:
    bn_stats/bn_aggr on the DVE produce mean/var per partition, the affine
    is a per-partition scalar-engine activation, and the per-feature
    gamma/beta are elementwise with partition-broadcast rows.  No stats
    matmuls, no broadcast roundtrips, no activation spills, and the
    cell/hidden state updates plus output stores need no transposes.
"""

import sys
from contextlib import ExitStack

import numpy as np

sys.path.insert(0, "/opt/trn_rl_repo")

import concourse.bass as bass
import concourse.tile as tile
from concourse import bacc, mybir
from concourse.bass_utils import run_bass_kernel_spmd
from concourse.masks import make_identity

F32 = mybir.dt.float32
BF16 = mybir.dt.bfloat16
AF = mybir.ActivationFunctionType
ALU = mybir.AluOpType

B, CIN, H = 8192, 512, 2048
NCORES = 8
BC = B // NCORES            # 1024 batch rows per core
NB = BC // 128              # 8 batch chunks
H2 = 2 * H                  # 4096
KC = H2 // 128              # 32 contraction chunks for gate matmuls
PC = CIN // 128             # 4 contraction chunks for the input projection
FC = H // 128               # 16 feature chunks (feature-major comb halves)
SW = 4                      # f sweeps per gate
FS = H // SW                # 512 features per sweep (= 1 PSUM bank)
NHB = BC // 512             # 2 PSUM batch halves for the projection

GATES = ("c2", "i", "f", "o")
GATE_FUNC = {"f": AF.Sigmoid, "i": AF.Sigmoid, "c2": AF.Tanh, "o": AF.Sigmoid}
# z-tile tag ring: c2/f share one set of buffers, i/o the other.
ZTAG = {"c2": "zE", "i": "zO", "f": "zE", "o": "zO"}
NEXT_GATE = {"c2": "i", "i": "f", "f": "o", "o": None}


def _row(ap):
    """View a 1-D [N] DRAM AP as [1, N]."""
    return ap.rearrange("(o k) -> o k", o=1)


def _bcast_row(row_ap, parts=128):
    """Partition-broadcast view of a [1, N] DRAM AP."""
    return bass.AP(
        tensor=row_ap.tensor,
        offset=row_ap.offset,
        ap=[[0, parts]] + [list(d) for d in row_ap.ap[1:]],
    )


def build_kernel(nc):
    ins = {}

    def din(name, shape):
        ins[name] = nc.dram_tensor(name, shape, F32, kind="ExternalInput").ap()

    din("x", (BC, 1, CIN))
    din("h", (BC, H))
    din("c", (BC, H))
    din("W_proj", (H, CIN))
    din("b_proj", (H,))
    din("g_ln", (H2,))
    din("b_ln", (H2,))
    din("g_cn", (H,))
    din("b_cn", (H,))
    din("g_hn", (H,))
    din("b_hn", (H,))
    for g in GATES:
        din(f"W_{g}", (H, H2))
        din(f"b_{g}", (H,))
        din(f"g_{g}", (H,))
        din(f"beta_{g}", (H,))

    out_h = nc.dram_tensor("out_h", (BC, H), F32, kind="ExternalOutput").ap()
    out_c = nc.dram_tensor("out_c", (BC, H), F32, kind="ExternalOutput").ap()

    with tile.TileContext(nc) as tc, ExitStack() as ctx:
        build_body(ctx, tc, ins, out_h, out_c)
    nc.compile()
    return nc


def build_body(ctx, tc, ins, out_h, out_c):
    nc = tc.nc

    # ---------------- deep pools (live through gates and tail) ------------
    singles = ctx.enter_context(tc.tile_pool(name="singles", bufs=1))
    smallp = ctx.enter_context(tc.tile_pool(name="smallp", bufs=1))
    tscr = ctx.enter_context(tc.tile_pool(name="tscr", bufs=1))
    cpool = ctx.enter_context(tc.tile_pool(name="cpool", bufs=1))
    bnp = ctx.enter_context(tc.tile_pool(name="bnp", bufs=1))
    dram = ctx.enter_context(tc.tile_pool(name="dram", bufs=1, space="DRAM"))

    combp = tc.alloc_tile_pool(name="comb", bufs=1)
    comb = [combp.tile([128, BC], BF16, name=f"comb{k}", tag=f"comb{k}")
            for k in range(KC)]

    ident = singles.tile([128, 128], F32)
    make_identity(nc, ident)
    ones_bf = singles.tile([128, 1], BF16)
    nc.vector.memset(ones_bf, 1.0)
    eps_col = singles.tile([128, 1], F32)
    nc.vector.memset(eps_col, 1e-5)
    eps_row = singles.tile([1, 1], F32)
    nc.vector.memset(eps_row, 1e-5)

    cols_req = []

    def load_cols(name, n):
        # Placeholder tile; filled in prep via a contiguous load + PE
        # transpose (a strided [p, c] DMA would head-of-line block the ring).
        t = singles.tile([128, n], F32, name=f"cols_{name}")
        cols_req.append((name, n, t))
        return t

    g_ln = load_cols("g_ln", KC)
    b_ln = load_cols("b_ln", KC)
    b_proj = load_cols("b_proj", FC)

    # ---- weight casts fp32 -> bf16, DRAM -> DRAM on the SWDGE path -------
    # Only gate c2's weights are cast upfront; each later gate's casts are
    # emitted during the previous gate so the gpsimd DMA queue stays short
    # for the per-gate bias/gamma/beta row loads.
    wbf = {g: dram.tile([H, H2], BF16, name=f"wbf_{g}") for g in GATES}

    def emit_wcast(g, triggers):
        """Cast W_g to bf16 in DRAM, chunked per sweep.  The Tile scheduler
        is dependency-driven (emission order alone cannot delay an
        instruction), so each chunk is gated behind a trigger tile via a
        tiny overlapping write: tiny waits for the trigger's producer, the
        big cast overlaps the tiny's destination (WAW) and thus follows it.
        Without this the casts all start at t=0 and saturate the 16 SDMA
        engines exactly when the small prep loads need them."""
        n = len(triggers) if len(triggers) > 2 else SW
        rows = H // n
        for s in range(n):
            trig = triggers[s % len(triggers)]
            nc.gpsimd.dma_start(out=wbf[g][s * rows:s * rows + 1, 0:1],
                                in_=trig[0:1, 0:1])
            nc.gpsimd.dma_start(out=wbf[g][bass.ts(s, rows), :],
                                in_=ins[f"W_{g}"][bass.ts(s, rows), :])

    # ---------------- prep: x^T, h^T, W_proj^T, proj, concat-LN -----------
    prep = tc.alloc_tile_pool(name="prep", bufs=1)
    ppsum = tc.alloc_tile_pool(name="ppsum", bufs=1, space="PSUM")

    def transpose_chunk(src_ap, dst_ap):
        pt = ppsum.tile([128, 128], F32, tag="tp", bufs=2)
        nc.tensor.transpose(pt, src_ap, ident)
        nc.vector.tensor_copy(out=dst_ap, in_=pt)

    # per-partition constant columns: contiguous [n, 128] load + PE transpose
    for name, n, t in cols_req:
        raw = prep.tile([n, 128], F32, name=f"raw_{name}", tag="colraw",
                        bufs=3)
        nc.scalar.dma_start(out=raw,
                            in_=ins[name].rearrange("(c p) -> c p", p=128))
        pt = ppsum.tile([128, KC], F32, tag="cpt", bufs=1)
        nc.tensor.transpose(pt[:, :n], raw, ident[:n, :n])
        nc.vector.tensor_copy(out=t, in_=pt[:, :n])

    # Stage loads split across both HWDGE rings (issue rate is the prep
    # bottleneck): h rows (1MB each) on sync, x/W_proj/cols on scalar.
    xT = [prep.tile([128, BC], BF16, name=f"xT{j}", tag=f"xT{j}")
          for j in range(PC)]
    x2d = ins["x"].rearrange("b one k -> (b one) k")
    trig_hs = None
    for bt in range(NB):
        hs = prep.tile([128, H], F32, tag="hstage", bufs=3)
        nc.sync.dma_start(out=hs, in_=ins["h"][bass.ts(bt, 128), :])
        if bt == 1:
            trig_hs = hs
        xs = prep.tile([128, CIN], F32, tag="xstage", bufs=3)
        nc.scalar.dma_start(out=xs, in_=x2d[bass.ts(bt, 128), :])
        for j in range(PC):
            transpose_chunk(xs[:, bass.ts(j, 128)], xT[j][:, bass.ts(bt, 128)])
        for j in range(FC):
            transpose_chunk(hs[:, bass.ts(j, 128)],
                            comb[FC + j][:, bass.ts(bt, 128)])

    wpT = [prep.tile([128, H], BF16, name=f"wpT{j}", tag=f"wpT{j}")
           for j in range(PC)]
    trig_ws = None
    for f in range(FC):
        ws = prep.tile([128, CIN], F32, tag="wpstage", bufs=4)
        nc.scalar.dma_start(out=ws, in_=ins["W_proj"][bass.ts(f, 128), :])
        if f == 3:
            trig_ws = ws
        for j in range(PC):
            transpose_chunk(ws[:, bass.ts(j, 128)], wpT[j][:, bass.ts(f, 128)])

    # Gate c2's and i's weight casts start once early prep stage loads have
    # landed — they finish before their gates' XBAR streams need them, and
    # gates f/o's casts are triggered off the c2/i weight streams so only
    # a modest cast tail overlaps the XBAR traffic.
    emit_wcast("c2", [trig_hs, trig_ws])
    emit_wcast("i", [trig_ws, trig_hs])

    # xp^T = W_proj @ x^T + b_proj, feature-major into comb[0..FC)
    for f in range(FC):
        pj = [ppsum.tile([128, 512], F32, name=f"pj{f}_{hb}",
                         tag=f"pj{f % 2}_{hb}", bufs=1)
              for hb in range(NHB)]
        for j in range(PC):
            for hb in range(NHB):
                nc.tensor.matmul(pj[hb], wpT[j][:, bass.ts(f, 128)],
                                 xT[j][:, bass.ts(hb, 512)],
                                 start=(j == 0), stop=(j == PC - 1))
        for hb in range(NHB):
            nc.vector.tensor_scalar_add(out=comb[f][:, bass.ts(hb, 512)],
                                        in0=pj[hb], scalar1=b_proj[:, f:f + 1])

    # concat-LN stats: per-batch sum(z), sum(z^2) via ones-matmuls into one
    # PSUM bank (quadrant rows 0/32/64/96).
    ROFF = (0, 32, 64, 96)
    cstat = ppsum.tile([128, 512], F32, tag="stats")
    for k in range(KC):
        for hb in range(NHB):
            zs = comb[k][:, bass.ts(hb, 512)]
            sq = prep.tile([128, 512], BF16, tag="sq", bufs=2)
            nc.scalar.square(sq, zs)
            r0, r1 = ROFF[2 * hb], ROFF[2 * hb + 1]
            nc.tensor.matmul(cstat[r0:r0 + 1, :], ones_bf, zs,
                             start=(k == 0), stop=(k == KC - 1),
                             tile_position=(0, r0))
            nc.tensor.matmul(cstat[r1:r1 + 1, :], ones_bf, sq,
                             start=(k == 0), stop=(k == KC - 1),
                             tile_position=(0, r1))

    m = prep.tile([1, BC], F32, tag="mrow")
    v = prep.tile([1, BC], F32, tag="vrow")
    msq = prep.tile([1, BC], F32, tag="msqrow")
    for hb in range(NHB):
        s = bass.ts(hb, 512)
        r0, r1 = ROFF[2 * hb], ROFF[2 * hb + 1]
        nc.vector.tensor_scalar_mul(m[:, s], cstat[r0:r0 + 1, :], 1.0 / H2)
        nc.vector.tensor_scalar_mul(v[:, s], cstat[r1:r1 + 1, :], 1.0 / H2)
    nc.vector.tensor_mul(msq, m, m)
    nc.vector.tensor_sub(v, v, msq)                       # var
    nc.scalar.activation(out=v, in_=v, func=AF.Sqrt, bias=eps_row, scale=1.0)
    nc.vector.reciprocal(out=v, in_=v)                    # rstd
    nc.vector.tensor_mul(msq, m, v)
    nc.vector.tensor_scalar_mul(msq, msq, -1.0)           # -mean*rstd
    # Broadcast across partitions via a DRAM roundtrip on the HWDGE rings
    # (gpsimd's Q7 is mid cast-descriptor issuance here, so a
    # partition_broadcast would land late — measured slower).
    a_bc = prep.tile([128, BC], F32, tag="abc")
    c_bc = prep.tile([128, BC], F32, tag="cbc")
    for row, bc in ((v, a_bc), (msq, c_bc)):
        drow = dram.tile([1, BC], F32, name="drow", tag="drow", bufs=4)
        nc.sync.dma_start(out=drow, in_=row)
        nc.sync.dma_start(out=bc, in_=_bcast_row(drow))
    for k in range(KC):
        # Alternate the elementwise normalization between DVE and gpsimd —
        # this chain gates the first matmuls and DVE alone is the wall here.
        eng = nc.vector if k % 2 == 0 else nc.gpsimd
        t = prep.tile([128, BC], F32, tag="apply", bufs=4)
        eng.tensor_mul(t, comb[k], a_bc)
        eng.tensor_add(t, t, c_bc)
        nc.scalar.activation(out=comb[k], in_=t, func=AF.Tanh,
                             scale=g_ln[:, k:k + 1], bias=b_ln[:, k:k + 1])

    ppsum.release()
    prep.release()

    # ---------------- gates: batch-major z = comb^T @ W^T ------------------
    zpool = tc.alloc_tile_pool(name="zpool", bufs=1)
    wtp = tc.alloc_tile_pool(name="wtp", bufs=1)
    vbc = tc.alloc_tile_pool(name="vbc", bufs=1)
    gpsum = tc.alloc_tile_pool(name="gpsum", bufs=1, space="PSUM")

    def bcast_vec(pool, name, tag):
        """[H] DRAM fp32 row -> [128, H] bf16 partition-broadcast tile."""
        row = pool.tile([1, H], BF16, name=f"row_{name}", tag="vrow", bufs=1)
        nc.gpsimd.dma_start(out=row, in_=_row(ins[name]))  # cast f32->bf16
        full = pool.tile([128, H], BF16, name=f"bc_{name}", tag=tag, bufs=1)
        nc.gpsimd.partition_broadcast(full, row)
        return full

    def bm_norm_cols(bn_t):
        """bn groups -> (rstd, -mean*rstd) per-partition columns."""
        mv = smallp.tile([128, 2], F32, tag="mv", bufs=8)
        nc.vector.bn_aggr(mv, bn_t)
        sd = smallp.tile([128, 1], F32, tag="sd", bufs=8)
        nc.scalar.activation(out=sd, in_=mv[:, 1:2], func=AF.Sqrt,
                             bias=eps_col, scale=1.0)
        rstd = smallp.tile([128, 1], F32, tag="rstd", bufs=8)
        nc.vector.reciprocal(rstd, sd)
        negm = smallp.tile([128, 1], F32, tag="negm", bufs=8)
        nc.vector.tensor_scalar(out=negm, in0=mv[:, 0:1], scalar1=rstd,
                                scalar2=-1.0, op0=ALU.mult, op1=ALU.mult)
        return rstd, negm

    def bm_apply_slice(dst_ap, src_ap, rstd, negm, g_bc_s, b_bc_s, func):
        """dst = func(((src - m)*rstd)*g + b) on one [128, FS] slice."""
        t = tscr.tile([128, FS], BF16, tag="t", bufs=4)
        nc.scalar.activation(out=t, in_=src_ap, func=AF.Identity,
                             scale=rstd, bias=negm)
        nc.vector.tensor_mul(t, t, g_bc_s)
        nc.vector.tensor_add(t, t, b_bc_s)
        nc.scalar.activation(out=dst_ap, in_=t, func=func)

    zt = {}
    cp = [None] * NB
    bg_work = []        # deferred DVE/ACT closures, interleaved into sweeps
    for g in GATES:
        bb = bcast_vec(vbc, f"b_{g}", "bb")
        gg = bcast_vec(vbc, f"g_{g}", "gg")
        tb = bcast_vec(vbc, f"beta_{g}", "tb")

        z = [zpool.tile([128, H], BF16, name=f"z_{g}{b}", tag=f"{ZTAG[g]}{b}")
             for b in range(NB)]
        bn = [bnp.tile([128, 6 * SW], F32, name=f"bn_{g}{b}", tag=f"bn{b}",
                       bufs=2)
              for b in range(NB)]

        wt_sweep = []
        for s in range(SW):
            ps = [gpsum.tile([128, FS], F32, name=f"ps_{g}{s}_{b}",
                             tag=f"mm{b}", bufs=1)
                  for b in range(NB)]
            for k in range(KC):
                wt = wtp.tile([128, FS], BF16, tag="wt", bufs=12)
                nc.sync.dma_start_transpose(
                    wt, wbf[g][bass.ts(s, FS), bass.ts(k, 128)])
                if k in (0, KC // 2):
                    wt_sweep.append(wt)
                for b in range(NB):
                    nc.tensor.matmul(ps[b], comb[k][:, bass.ts(b, 128)], wt,
                                     start=(k == 0), stop=(k == KC - 1))
            for b in range(NB):
                # drain + bias (free-dim) in one DVE op, then stats
                zs = z[b][:, bass.ts(s, FS)]
                nc.vector.tensor_add(zs, ps[b], bb[:, bass.ts(s, FS)])
                nc.vector.bn_stats(out=bn[b][:, 6 * s:6 * (s + 1)], in_=zs)
            for _ in range(2):
                if bg_work:
                    bg_work.pop(0)()

        # Gate g+2's cast chunk s unblocks once this gate's sweep-s weight
        # stream is underway — casts run one full gate ahead of their
        # consumers, spread across sweeps.
        if g == "c2":
            emit_wcast("f", wt_sweep)
        elif g == "i":
            emit_wcast("o", wt_sweep)

        for b in range(NB):
            rstd, negm = bm_norm_cols(bn[b])
            for s in range(SW):
                sl = bass.ts(s, FS)
                bm_apply_slice(z[b][:, sl], z[b][:, sl], rstd, negm,
                               gg[:, sl], tb[:, sl], GATE_FUNC[g])

        zt[g] = z

        if g == "i":
            # cp = i * cc  (cc = gate c2 output, still resident)
            for b in range(NB):
                cp[b] = zpool.tile([128, H], BF16, name=f"cp{b}",
                                   tag=f"cp{b}")
                nc.vector.tensor_mul(cp[b], zt["i"][b], zt["c2"][b])
        elif g == "f":
            # cp += f * c, with c loaded batch-major (cast to bf16 in DMA);
            # then prefetch gate o's weight casts.
            for b in range(NB):
                ct = cpool.tile([128, H], BF16, tag="c", bufs=1)
                nc.gpsimd.dma_start(out=ct, in_=ins["c"][bass.ts(b, 128), :])
                for s in range(SW):
                    sl = bass.ts(s, FS)
                    t = tscr.tile([128, FS], BF16, tag="t", bufs=4)
                    nc.vector.tensor_mul(t, zt["f"][b][:, sl], ct[:, sl])
                    nc.vector.tensor_add(cp[b][:, sl], cp[b][:, sl], t)

            # Cell path: LN_cn(cp) -> out_c, then cp <- tanh(next_cell) in
            # place.  Deferred as closures so the work interleaves into gate
            # o's sweep loop: pure DVE/ACT/DMA that executes while gate o's
            # matmuls occupy the PE.  out_c is written through a bf16
            # cast-DMA (SWDGE) to avoid fp32 staging during the gate window.
            g_cn = bcast_vec(vbc, "g_cn", "g_cn")
            b_cn = bcast_vec(vbc, "b_cn", "b_cn")

            def cell_work(b):
                bn_c = bnp.tile([128, 6 * SW], F32, name=f"bnc{b}",
                                tag=f"bn{b}", bufs=2)
                for s in range(SW):
                    nc.vector.bn_stats(out=bn_c[:, 6 * s:6 * (s + 1)],
                                       in_=cp[b][:, bass.ts(s, FS)])
                rstd, negm = bm_norm_cols(bn_c)
                for s in range(SW):
                    sl = bass.ts(s, FS)
                    t = tscr.tile([128, FS], BF16, tag="t", bufs=4)
                    nc.scalar.activation(out=t, in_=cp[b][:, sl],
                                         func=AF.Identity,
                                         scale=rstd, bias=negm)
                    nc.vector.tensor_mul(t, t, g_cn[:, sl])
                    nc.vector.tensor_add(t, t, b_cn[:, sl])
                    nc.gpsimd.dma_start(out=out_c[bass.ts(b, 128), sl],
                                        in_=t)  # bf16 -> fp32 cast store
                    nc.scalar.activation(out=cp[b][:, sl], in_=t,
                                         func=AF.Tanh)

            bg_work.extend(
                (lambda b=b: cell_work(b)) for b in range(NB))

    while bg_work:
        bg_work.pop(0)()

    gpsum.release()
    vbc.release()
    wtp.release()

    # ---------------- tail: cell LN, hidden path, outputs ------------------
    tailp = tc.alloc_tile_pool(name="tailp", bufs=1)

    def bcast_tail(name):
        row = tailp.tile([1, H], BF16, name=f"row_{name}", tag="trow", bufs=1)
        nc.gpsimd.dma_start(out=row, in_=_row(ins[name]))
        full = tailp.tile([128, H], BF16, name=f"bc_{name}", tag=name, bufs=1)
        nc.gpsimd.partition_broadcast(full, row)
        return full

    g_hn = bcast_tail("g_hn")
    b_hn = bcast_tail("b_hn")

    for b in range(NB):
        # hidden: hp = o * tanh(next_cell) (cp holds the tanh), LN_hn + tanh
        hp = zt["o"][b]
        nc.vector.tensor_mul(hp, hp, cp[b])
        bn_h = bnp.tile([128, 6 * SW], F32, tag=f"bn{b}", bufs=2)
        for s in range(SW):
            nc.vector.bn_stats(out=bn_h[:, 6 * s:6 * (s + 1)],
                               in_=hp[:, bass.ts(s, FS)])
        rstd, negm = bm_norm_cols(bn_h)
        t = tailp.tile([128, H], BF16, tag="ttail", bufs=2)
        nc.scalar.activation(out=t, in_=hp, func=AF.Identity,
                             scale=rstd, bias=negm)
        nc.vector.tensor_mul(t, t, g_hn)
        nc.vector.tensor_add(t, t, b_hn)
        t2 = tailp.tile([128, H], BF16, tag="ttail2", bufs=2)
        nc.scalar.activation(out=t2, in_=t, func=AF.Tanh)
        nc.gpsimd.dma_start(out=out_h[bass.ts(b, 128), :], in_=t2)

    tailp.release()
    zpool.release()
    combp.release()


_NC_CACHE = {}


def _get_nc():
    if "nc" not in _NC_CACHE:
        nc = bacc.Bacc(
            "TRN2",
            target_bir_lowering=False,
            debug=False,
            enable_asserts=False,
            num_devices=NCORES,
        )
        _NC_CACHE["nc"] = build_kernel(nc)
    return _NC_CACHE["nc"]


def run(inputs, **kw):
    nc = _get_nc()
    full = {k: np.ascontiguousarray(np.asarray(v, dtype=np.float32))
            for k, v in inputs.items()}
    in_maps = []
    for i in range(NCORES):
        s = slice(i * BC, (i + 1) * BC)
        m = {k: (np.ascontiguousarray(v[s]) if k in ("x", "h", "c") else v)
             for k, v in full.items()}
        in_maps.append(m)
    res = run_bass_kernel_spmd(nc, in_maps, core_ids=list(range(NCORES)), **kw)
    nh = np.concatenate([r["out_h"] for r in res.results], axis=0)
    ncl = np.concatenate([r["out_c"] for r in res.results], axis=0)
    return np.stack([nh, ncl]).astype(np.float32), res


def kernel(**inputs) -> np.ndarray:
    out, _ = run(inputs)
    return out


# revision 48
# speedup vs baseline: 1.0511x; 1.0511x over previous
"""Trainium2 Bass kernel for a custom LSTM cell with LayerNorms.

Data-parallel across 8 NeuronCores: batch B=8192 is split into 8 shards of
1024 rows; weights are replicated.

Dataflow (v2):
  - comb = tanh(LN([x W_proj^T ; h])) is built feature-major ([feature,
    batch] tiles) exactly once: x/h/W_proj are transposed on the PE, the
    concat-LN statistics are ones-vector matmuls accumulated in one PSUM
    bank, and the mean/rstd rows are broadcast via a DRAM roundtrip.
  - The four gate matmuls produce BATCH-major outputs: the stationary
    operand is a [128k, 128b] slice of comb, the moving operand is a
    [128k, 512f] slice of W^T obtained by XBAR DMA-transpose from a bf16
    copy of W (written once by a fp32->bf16 cast-during-DMA on the SWDGE
    path, chunked and emitted one gate ahead so casts overlap matmuls).
    k is the outer loop so all 8 batch-chunk PSUM banks accumulate in
    parallel and only a handful of W^T tiles are resident.
  - Batch-major layout makes every per-batch LayerNorm a free-dim problem:
    bn_stats/bn_aggr on the DVE produce mean/var per partition, the affine
    is a per-partition scalar-engine activation, and the per-feature
    gamma/beta are elementwise with partition-broadcast rows.  No stats
    matmuls, no broadcast roundtrips, no activation spills, and the
    cell/hidden state updates plus output stores need no transposes.
"""

import sys
from contextlib import ExitStack

import numpy as np

sys.path.insert(0, "/opt/trn_rl_repo")

import concourse.bass as bass
import concourse.tile as tile
from concourse import bacc, mybir
from concourse.bass_utils import run_bass_kernel_spmd
from concourse.masks import make_identity

F32 = mybir.dt.float32
BF16 = mybir.dt.bfloat16
AF = mybir.ActivationFunctionType
ALU = mybir.AluOpType

B, CIN, H = 8192, 512, 2048
NCORES = 8
BC = B // NCORES            # 1024 batch rows per core
NB = BC // 128              # 8 batch chunks
H2 = 2 * H                  # 4096
KC = H2 // 128              # 32 contraction chunks for gate matmuls
PC = CIN // 128             # 4 contraction chunks for the input projection
FC = H // 128               # 16 feature chunks (feature-major comb halves)
SW = 4                      # f sweeps per gate
FS = H // SW                # 512 features per sweep (= 1 PSUM bank)
NHB = BC // 512             # 2 PSUM batch halves for the projection

GATES = ("c2", "i", "f", "o")
GATE_FUNC = {"f": AF.Sigmoid, "i": AF.Sigmoid, "c2": AF.Tanh, "o": AF.Sigmoid}
# z-tile tag ring: c2/f share one set of buffers, i/o the other.
ZTAG = {"c2": "zE", "i": "zO", "f": "zE", "o": "zO"}
NEXT_GATE = {"c2": "i", "i": "f", "f": "o", "o": None}


def _row(ap):
    """View a 1-D [N] DRAM AP as [1, N]."""
    return ap.rearrange("(o k) -> o k", o=1)


def _bcast_row(row_ap, parts=128):
    """Partition-broadcast view of a [1, N] DRAM AP."""
    return bass.AP(
        tensor=row_ap.tensor,
        offset=row_ap.offset,
        ap=[[0, parts]] + [list(d) for d in row_ap.ap[1:]],
    )


def build_kernel(nc):
    ins = {}

    def din(name, shape):
        ins[name] = nc.dram_tensor(name, shape, F32, kind="ExternalInput").ap()

    din("x", (BC, 1, CIN))
    din("h", (BC, H))
    din("c", (BC, H))
    din("W_proj", (H, CIN))
    din("b_proj", (H,))
    din("g_ln", (H2,))
    din("b_ln", (H2,))
    din("g_cn", (H,))
    din("b_cn", (H,))
    din("g_hn", (H,))
    din("b_hn", (H,))
    for g in GATES:
        din(f"W_{g}", (H, H2))
        din(f"b_{g}", (H,))
        din(f"g_{g}", (H,))
        din(f"beta_{g}", (H,))

    out_h = nc.dram_tensor("out_h", (BC, H), F32, kind="ExternalOutput").ap()
    out_c = nc.dram_tensor("out_c", (BC, H), F32, kind="ExternalOutput").ap()

    with tile.TileContext(nc) as tc, ExitStack() as ctx:
        build_body(ctx, tc, ins, out_h, out_c)
    nc.compile()
    return nc


def build_body(ctx, tc, ins, out_h, out_c):
    nc = tc.nc

    # ---------------- deep pools (live through gates and tail) ------------
    singles = ctx.enter_context(tc.tile_pool(name="singles", bufs=1))
    smallp = ctx.enter_context(tc.tile_pool(name="smallp", bufs=1))
    tscr = ctx.enter_context(tc.tile_pool(name="tscr", bufs=1))
    cpool = ctx.enter_context(tc.tile_pool(name="cpool", bufs=1))
    bnp = ctx.enter_context(tc.tile_pool(name="bnp", bufs=1))
    dram = ctx.enter_context(tc.tile_pool(name="dram", bufs=1, space="DRAM"))

    combp = tc.alloc_tile_pool(name="comb", bufs=1)
    comb = [combp.tile([128, BC], BF16, name=f"comb{k}", tag=f"comb{k}")
            for k in range(KC)]

    ident = singles.tile([128, 128], F32)
    make_identity(nc, ident)
    ones_bf = singles.tile([128, 1], BF16)
    nc.vector.memset(ones_bf, 1.0)
    eps_col = singles.tile([128, 1], F32)
    nc.vector.memset(eps_col, 1e-5)
    eps_row = singles.tile([1, 1], F32)
    nc.vector.memset(eps_row, 1e-5)

    cols_req = []

    def load_cols(name, n):
        # Placeholder tile; filled in prep via a contiguous load + PE
        # transpose (a strided [p, c] DMA would head-of-line block the ring).
        t = singles.tile([128, n], F32, name=f"cols_{name}")
        cols_req.append((name, n, t))
        return t

    g_ln = load_cols("g_ln", KC)
    b_ln = load_cols("b_ln", KC)
    b_proj = load_cols("b_proj", FC)

    # ---- weight casts fp32 -> bf16, DRAM -> DRAM on the SWDGE path -------
    # Only gate c2's weights are cast upfront; each later gate's casts are
    # emitted during the previous gate so the gpsimd DMA queue stays short
    # for the per-gate bias/gamma/beta row loads.
    wbf = {g: dram.tile([H, H2], BF16, name=f"wbf_{g}") for g in GATES}

    def emit_wcast(g, triggers):
        """Cast W_g to bf16 in DRAM, chunked per sweep.  The Tile scheduler
        is dependency-driven (emission order alone cannot delay an
        instruction), so each chunk is gated behind a trigger tile via a
        tiny overlapping write: tiny waits for the trigger's producer, the
        big cast overlaps the tiny's destination (WAW) and thus follows it.
        Without this the casts all start at t=0 and saturate the 16 SDMA
        engines exactly when the small prep loads need them."""
        n = len(triggers) if len(triggers) > 2 else SW
        rows = H // n
        for s in range(n):
            trig = triggers[s % len(triggers)]
            nc.gpsimd.dma_start(out=wbf[g][s * rows:s * rows + 1, 0:1],
                                in_=trig[0:1, 0:1])
            nc.gpsimd.dma_start(out=wbf[g][bass.ts(s, rows), :],
                                in_=ins[f"W_{g}"][bass.ts(s, rows), :])

    # ---------------- prep: x^T, h^T, W_proj^T, proj, concat-LN -----------
    prep = tc.alloc_tile_pool(name="prep", bufs=1)
    ppsum = tc.alloc_tile_pool(name="ppsum", bufs=1, space="PSUM")

    def transpose_chunk(src_ap, dst_ap):
        pt = ppsum.tile([128, 128], F32, tag="tp", bufs=2)
        nc.tensor.transpose(pt, src_ap, ident)
        nc.vector.tensor_copy(out=dst_ap, in_=pt)

    # per-partition constant columns: contiguous [n, 128] load + PE transpose
    for name, n, t in cols_req:
        raw = prep.tile([n, 128], F32, name=f"raw_{name}", tag="colraw",
                        bufs=3)
        nc.scalar.dma_start(out=raw,
                            in_=ins[name].rearrange("(c p) -> c p", p=128))
        pt = ppsum.tile([128, KC], F32, tag="cpt", bufs=1)
        nc.tensor.transpose(pt[:, :n], raw, ident[:n, :n])
        nc.vector.tensor_copy(out=t, in_=pt[:, :n])

    # Stage loads split across both HWDGE rings (issue rate is the prep
    # bottleneck): h rows (1MB each) on sync, x/W_proj/cols on scalar.
    xT = [prep.tile([128, BC], BF16, name=f"xT{j}", tag=f"xT{j}")
          for j in range(PC)]
    x2d = ins["x"].rearrange("b one k -> (b one) k")
    trig_hs = None
    for bt in range(NB):
        hs = prep.tile([128, H], F32, tag="hstage", bufs=3)
        nc.sync.dma_start(out=hs, in_=ins["h"][bass.ts(bt, 128), :])
        if bt == 1:
            trig_hs = hs
        xs = prep.tile([128, CIN], F32, tag="xstage", bufs=3)
        nc.scalar.dma_start(out=xs, in_=x2d[bass.ts(bt, 128), :])
        for j in range(PC):
            transpose_chunk(xs[:, bass.ts(j, 128)], xT[j][:, bass.ts(bt, 128)])
        for j in range(FC):
            transpose_chunk(hs[:, bass.ts(j, 128)],
                            comb[FC + j][:, bass.ts(bt, 128)])

    wpT = [prep.tile([128, H], BF16, name=f"wpT{j}", tag=f"wpT{j}")
           for j in range(PC)]
    trig_ws = None
    for f in range(FC):
        ws = prep.tile([128, CIN], F32, tag="wpstage", bufs=4)
        nc.scalar.dma_start(out=ws, in_=ins["W_proj"][bass.ts(f, 128), :])
        if f == 3:
            trig_ws = ws
        for j in range(PC):
            transpose_chunk(ws[:, bass.ts(j, 128)], wpT[j][:, bass.ts(f, 128)])

    # Gate c2's and i's weight casts start once early prep stage loads have
    # landed — they finish before their gates' XBAR streams need them, and
    # gates f/o's casts are triggered off the c2/i weight streams so only
    # a modest cast tail overlaps the XBAR traffic.
    emit_wcast("c2", [trig_hs, trig_ws])
    emit_wcast("i", [trig_ws, trig_hs])

    # xp^T = W_proj @ x^T + b_proj, feature-major into comb[0..FC)
    for f in range(FC):
        pj = [ppsum.tile([128, 512], F32, name=f"pj{f}_{hb}",
                         tag=f"pj{f % 2}_{hb}", bufs=1)
              for hb in range(NHB)]
        for j in range(PC):
            for hb in range(NHB):
                nc.tensor.matmul(pj[hb], wpT[j][:, bass.ts(f, 128)],
                                 xT[j][:, bass.ts(hb, 512)],
                                 start=(j == 0), stop=(j == PC - 1))
        for hb in range(NHB):
            nc.vector.tensor_scalar_add(out=comb[f][:, bass.ts(hb, 512)],
                                        in0=pj[hb], scalar1=b_proj[:, f:f + 1])

    # concat-LN stats: per-batch sum(z), sum(z^2) via ones-matmuls into one
    # PSUM bank (quadrant rows 0/32/64/96).
    ROFF = (0, 32, 64, 96)
    cstat = ppsum.tile([128, 512], F32, tag="stats")
    for k in range(KC):
        for hb in range(NHB):
            zs = comb[k][:, bass.ts(hb, 512)]
            sq = prep.tile([128, 512], BF16, tag="sq", bufs=2)
            nc.scalar.square(sq, zs)
            r0, r1 = ROFF[2 * hb], ROFF[2 * hb + 1]
            nc.tensor.matmul(cstat[r0:r0 + 1, :], ones_bf, zs,
                             start=(k == 0), stop=(k == KC - 1),
                             tile_position=(0, r0))
            nc.tensor.matmul(cstat[r1:r1 + 1, :], ones_bf, sq,
                             start=(k == 0), stop=(k == KC - 1),
                             tile_position=(0, r1))

    m = prep.tile([1, BC], F32, tag="mrow")
    v = prep.tile([1, BC], F32, tag="vrow")
    msq = prep.tile([1, BC], F32, tag="msqrow")
    for hb in range(NHB):
        s = bass.ts(hb, 512)
        r0, r1 = ROFF[2 * hb], ROFF[2 * hb + 1]
        nc.vector.tensor_scalar_mul(m[:, s], cstat[r0:r0 + 1, :], 1.0 / H2)
        nc.vector.tensor_scalar_mul(v[:, s], cstat[r1:r1 + 1, :], 1.0 / H2)
    nc.vector.tensor_mul(msq, m, m)
    nc.vector.tensor_sub(v, v, msq)                       # var
    nc.scalar.activation(out=v, in_=v, func=AF.Sqrt, bias=eps_row, scale=1.0)
    nc.vector.reciprocal(out=v, in_=v)                    # rstd
    nc.vector.tensor_mul(msq, m, v)
    nc.vector.tensor_scalar_mul(msq, msq, -1.0)           # -mean*rstd
    # Broadcast across partitions via a DRAM roundtrip on the HWDGE rings
    # (gpsimd's Q7 is mid cast-descriptor issuance here, so a
    # partition_broadcast would land late — measured slower).
    a_bc = prep.tile([128, BC], F32, tag="abc")
    c_bc = prep.tile([128, BC], F32, tag="cbc")
    for row, bc in ((v, a_bc), (msq, c_bc)):
        drow = dram.tile([1, BC], F32, name="drow", tag="drow", bufs=4)
        nc.sync.dma_start(out=drow, in_=row)
        nc.sync.dma_start(out=bc, in_=_bcast_row(drow))
    for k in range(KC):
        t = prep.tile([128, BC], F32, tag="apply", bufs=4)
        nc.vector.tensor_mul(t, comb[k], a_bc)
        nc.vector.tensor_add(t, t, c_bc)
        nc.scalar.activation(out=comb[k], in_=t, func=AF.Tanh,
                             scale=g_ln[:, k:k + 1], bias=b_ln[:, k:k + 1])

    ppsum.release()
    prep.release()

    # ---------------- gates: batch-major z = comb^T @ W^T ------------------
    zpool = tc.alloc_tile_pool(name="zpool", bufs=1)
    wtp = tc.alloc_tile_pool(name="wtp", bufs=1)
    vbc = tc.alloc_tile_pool(name="vbc", bufs=1)
    gpsum = tc.alloc_tile_pool(name="gpsum", bufs=1, space="PSUM")

    def bcast_vec(pool, name, tag):
        """[H] DRAM fp32 row -> [128, H] bf16 partition-broadcast tile."""
        row = pool.tile([1, H], BF16, name=f"row_{name}", tag="vrow", bufs=1)
        nc.gpsimd.dma_start(out=row, in_=_row(ins[name]))  # cast f32->bf16
        full = pool.tile([128, H], BF16, name=f"bc_{name}", tag=tag, bufs=1)
        nc.gpsimd.partition_broadcast(full, row)
        return full

    def bm_norm_cols(bn_t):
        """bn groups -> (rstd, -mean*rstd) per-partition columns."""
        mv = smallp.tile([128, 2], F32, tag="mv", bufs=8)
        nc.vector.bn_aggr(mv, bn_t)
        sd = smallp.tile([128, 1], F32, tag="sd", bufs=8)
        nc.scalar.activation(out=sd, in_=mv[:, 1:2], func=AF.Sqrt,
                             bias=eps_col, scale=1.0)
        rstd = smallp.tile([128, 1], F32, tag="rstd", bufs=8)
        nc.vector.reciprocal(rstd, sd)
        negm = smallp.tile([128, 1], F32, tag="negm", bufs=8)
        nc.vector.tensor_scalar(out=negm, in0=mv[:, 0:1], scalar1=rstd,
                                scalar2=-1.0, op0=ALU.mult, op1=ALU.mult)
        return rstd, negm

    def bm_apply_slice(dst_ap, src_ap, rstd, negm, g_bc_s, b_bc_s, func):
        """dst = func(((src - m)*rstd)*g + b) on one [128, FS] slice."""
        t = tscr.tile([128, FS], BF16, tag="t", bufs=4)
        nc.scalar.activation(out=t, in_=src_ap, func=AF.Identity,
                             scale=rstd, bias=negm)
        nc.vector.tensor_mul(t, t, g_bc_s)
        nc.vector.tensor_add(t, t, b_bc_s)
        nc.scalar.activation(out=dst_ap, in_=t, func=func)

    zt = {}
    cp = [None] * NB
    bg_work = []        # deferred DVE/ACT closures, interleaved into sweeps
    for g in GATES:
        bb = bcast_vec(vbc, f"b_{g}", "bb")
        gg = bcast_vec(vbc, f"g_{g}", "gg")
        tb = bcast_vec(vbc, f"beta_{g}", "tb")

        z = [zpool.tile([128, H], BF16, name=f"z_{g}{b}", tag=f"{ZTAG[g]}{b}")
             for b in range(NB)]
        bn = [bnp.tile([128, 6 * SW], F32, name=f"bn_{g}{b}", tag=f"bn{b}",
                       bufs=2)
              for b in range(NB)]

        wt_sweep = []
        for s in range(SW):
            ps = [gpsum.tile([128, FS], F32, name=f"ps_{g}{s}_{b}",
                             tag=f"mm{b}", bufs=1)
                  for b in range(NB)]
            for k in range(KC):
                wt = wtp.tile([128, FS], BF16, tag="wt", bufs=12)
                nc.sync.dma_start_transpose(
                    wt, wbf[g][bass.ts(s, FS), bass.ts(k, 128)])
                if k in (0, KC // 2):
                    wt_sweep.append(wt)
                for b in range(NB):
                    nc.tensor.matmul(ps[b], comb[k][:, bass.ts(b, 128)], wt,
                                     start=(k == 0), stop=(k == KC - 1))
            for b in range(NB):
                # drain + bias (free-dim) in one DVE op, then stats
                zs = z[b][:, bass.ts(s, FS)]
                nc.vector.tensor_add(zs, ps[b], bb[:, bass.ts(s, FS)])
                nc.vector.bn_stats(out=bn[b][:, 6 * s:6 * (s + 1)], in_=zs)
            for _ in range(2):
                if bg_work:
                    bg_work.pop(0)()

        # Gate g+2's cast chunk s unblocks once this gate's sweep-s weight
        # stream is underway — casts run one full gate ahead of their
        # consumers, spread across sweeps.
        if g == "c2":
            emit_wcast("f", wt_sweep)
        elif g == "i":
            emit_wcast("o", wt_sweep)

        for b in range(NB):
            rstd, negm = bm_norm_cols(bn[b])
            for s in range(SW):
                sl = bass.ts(s, FS)
                bm_apply_slice(z[b][:, sl], z[b][:, sl], rstd, negm,
                               gg[:, sl], tb[:, sl], GATE_FUNC[g])

        zt[g] = z

        if g == "i":
            # cp = i * cc  (cc = gate c2 output, still resident)
            for b in range(NB):
                cp[b] = zpool.tile([128, H], BF16, name=f"cp{b}",
                                   tag=f"cp{b}")
                nc.vector.tensor_mul(cp[b], zt["i"][b], zt["c2"][b])
        elif g == "f":
            # cp += f * c, with c loaded batch-major (cast to bf16 in DMA);
            # then prefetch gate o's weight casts.
            for b in range(NB):
                ct = cpool.tile([128, H], BF16, tag="c", bufs=1)
                nc.gpsimd.dma_start(out=ct, in_=ins["c"][bass.ts(b, 128), :])
                for s in range(SW):
                    sl = bass.ts(s, FS)
                    t = tscr.tile([128, FS], BF16, tag="t", bufs=4)
                    nc.vector.tensor_mul(t, zt["f"][b][:, sl], ct[:, sl])
                    nc.vector.tensor_add(cp[b][:, sl], cp[b][:, sl], t)

            # Cell path: LN_cn(cp) -> out_c, then cp <- tanh(next_cell) in
            # place.  Deferred as closures so the work interleaves into gate
            # o's sweep loop: pure DVE/ACT/DMA that executes while gate o's
            # matmuls occupy the PE.  out_c is written through a bf16
            # cast-DMA (SWDGE) to avoid fp32 staging during the gate window.
            g_cn = bcast_vec(vbc, "g_cn", "g_cn")
            b_cn = bcast_vec(vbc, "b_cn", "b_cn")

            def cell_work(b):
                bn_c = bnp.tile([128, 6 * SW], F32, name=f"bnc{b}",
                                tag=f"bn{b}", bufs=2)
                for s in range(SW):
                    nc.vector.bn_stats(out=bn_c[:, 6 * s:6 * (s + 1)],
                                       in_=cp[b][:, bass.ts(s, FS)])
                rstd, negm = bm_norm_cols(bn_c)
                for s in range(SW):
                    sl = bass.ts(s, FS)
                    t = tscr.tile([128, FS], BF16, tag="t", bufs=4)
                    nc.scalar.activation(out=t, in_=cp[b][:, sl],
                                         func=AF.Identity,
                                         scale=rstd, bias=negm)
                    nc.vector.tensor_mul(t, t, g_cn[:, sl])
                    nc.vector.tensor_add(t, t, b_cn[:, sl])
                    nc.gpsimd.dma_start(out=out_c[bass.ts(b, 128), sl],
                                        in_=t)  # bf16 -> fp32 cast store
                    nc.scalar.activation(out=cp[b][:, sl], in_=t,
                                         func=AF.Tanh)

            bg_work.extend(
                (lambda b=b: cell_work(b)) for b in range(NB))

    while bg_work:
        bg_work.pop(0)()

    gpsum.release()
    vbc.release()
    wtp.release()

    # ---------------- tail: cell LN, hidden path, outputs ------------------
    tailp = tc.alloc_tile_pool(name="tailp", bufs=1)

    def bcast_tail(name):
        row = tailp.tile([1, H], BF16, name=f"row_{name}", tag="trow", bufs=1)
        nc.gpsimd.dma_start(out=row, in_=_row(ins[name]))
        full = tailp.tile([128, H], BF16, name=f"bc_{name}", tag=name, bufs=1)
        nc.gpsimd.partition_broadcast(full, row)
        return full

    g_hn = bcast_tail("g_hn")
    b_hn = bcast_tail("b_hn")

    for b in range(NB):
        # hidden: hp = o * tanh(next_cell) (cp holds the tanh), LN_hn + tanh
        hp = zt["o"][b]
        nc.vector.tensor_mul(hp, hp, cp[b])
        bn_h = bnp.tile([128, 6 * SW], F32, tag=f"bn{b}", bufs=2)
        for s in range(SW):
            nc.vector.bn_stats(out=bn_h[:, 6 * s:6 * (s + 1)],
                               in_=hp[:, bass.ts(s, FS)])
        rstd, negm = bm_norm_cols(bn_h)
        t = tailp.tile([128, H], BF16, tag="ttail", bufs=2)
        nc.scalar.activation(out=t, in_=hp, func=AF.Identity,
                             scale=rstd, bias=negm)
        nc.vector.tensor_mul(t, t, g_hn)
        nc.vector.tensor_add(t, t, b_hn)
        t2 = tailp.tile([128, H], BF16, tag="ttail2", bufs=2)
        nc.scalar.activation(out=t2, in_=t, func=AF.Tanh)
        nc.gpsimd.dma_start(out=out_h[bass.ts(b, 128), :], in_=t2)

    tailp.release()
    zpool.release()
    combp.release()


_NC_CACHE = {}


def _get_nc():
    if "nc" not in _NC_CACHE:
        nc = bacc.Bacc(
            "TRN2",
            target_bir_lowering=False,
            debug=False,
            enable_asserts=False,
            num_devices=NCORES,
        )
        _NC_CACHE["nc"] = build_kernel(nc)
    return _NC_CACHE["nc"]


def run(inputs, **kw):
    nc = _get_nc()
    full = {k: np.ascontiguousarray(np.asarray(v, dtype=np.float32))
            for k, v in inputs.items()}
    in_maps = []
    for i in range(NCORES):
        s = slice(i * BC, (i + 1) * BC)
        m = {k: (np.ascontiguousarray(v[s]) if k in ("x", "h", "c") else v)
             for k, v in full.items()}
        in_maps.append(m)
    res = run_bass_kernel_spmd(nc, in_maps, core_ids=list(range(NCORES)), **kw)
    nh = np.concatenate([r["out_h"] for r in res.results], axis=0)
    ncl = np.concatenate([r["out_c"] for r in res.results], axis=0)
    return np.stack([nh, ncl]).astype(np.float32), res


def kernel(**inputs) -> np.ndarray:
    out, _ = run(inputs)
    return out


# revision 49
# speedup vs baseline: 1.0998x; 1.0463x over previous
"""Trainium2 Bass kernel for a custom LSTM cell with LayerNorms.

Data-parallel across 8 NeuronCores: batch B=8192 is split into 8 shards of
1024 rows; weights are replicated.

Dataflow (v2):
  - comb = tanh(LN([x W_proj^T ; h])) is built feature-major ([feature,
    batch] tiles) exactly once: x/h/W_proj are transposed on the PE, the
    concat-LN statistics are ones-vector matmuls accumulated in one PSUM
    bank, and the mean/rstd rows are broadcast via a DRAM roundtrip.
  - The four gate matmuls produce BATCH-major outputs: the stationary
    operand is a [128k, 128b] slice of comb, the moving operand is a
    [128k, 512f] slice of W^T obtained by XBAR DMA-transpose from a bf16
    copy of W (written once by a fp32->bf16 cast-during-DMA on the SWDGE
    path, chunked and emitted one gate ahead so casts overlap matmuls).
    k is the outer loop so all 8 batch-chunk PSUM banks accumulate in
    parallel and only a handful of W^T tiles are resident.
  - Batch-major layout makes every per-batch LayerNorm a free-dim problem:
    bn_stats/bn_aggr on the DVE produce mean/var per partition, the affine
    is a per-partition scalar-engine activation, and the per-feature
    gamma/beta are elementwise with partition-broadcast rows.  No stats
    matmuls, no broadcast roundtrips, no activation spills, and the
    cell/hidden state updates plus output stores need no transposes.
"""

import sys
from contextlib import ExitStack

import numpy as np

sys.path.insert(0, "/opt/trn_rl_repo")

import concourse.bass as bass
import concourse.tile as tile
from concourse import bacc, mybir
from concourse.bass_utils import run_bass_kernel_spmd
from concourse.masks import make_identity

F32 = mybir.dt.float32
BF16 = mybir.dt.bfloat16
AF = mybir.ActivationFunctionType
ALU = mybir.AluOpType

B, CIN, H = 8192, 512, 2048
NCORES = 8
BC = B // NCORES            # 1024 batch rows per core
NB = BC // 128              # 8 batch chunks
H2 = 2 * H                  # 4096
KC = H2 // 128              # 32 contraction chunks for gate matmuls
PC = CIN // 128             # 4 contraction chunks for the input projection
FC = H // 128               # 16 feature chunks (feature-major comb halves)
SW = 4                      # f sweeps per gate
FS = H // SW                # 512 features per sweep (= 1 PSUM bank)
NHB = BC // 512             # 2 PSUM batch halves for the projection

GATES = ("c2", "i", "f", "o")
GATE_FUNC = {"f": AF.Sigmoid, "i": AF.Sigmoid, "c2": AF.Tanh, "o": AF.Sigmoid}
# z-tile tag ring: c2/f share one set of buffers, i/o the other.
ZTAG = {"c2": "zE", "i": "zO", "f": "zE", "o": "zO"}
NEXT_GATE = {"c2": "i", "i": "f", "f": "o", "o": None}


def _row(ap):
    """View a 1-D [N] DRAM AP as [1, N]."""
    return ap.rearrange("(o k) -> o k", o=1)


def _bcast_row(row_ap, parts=128):
    """Partition-broadcast view of a [1, N] DRAM AP."""
    return bass.AP(
        tensor=row_ap.tensor,
        offset=row_ap.offset,
        ap=[[0, parts]] + [list(d) for d in row_ap.ap[1:]],
    )


def build_kernel(nc):
    ins = {}

    def din(name, shape):
        ins[name] = nc.dram_tensor(name, shape, F32, kind="ExternalInput").ap()

    din("x", (BC, 1, CIN))
    din("h", (BC, H))
    din("c", (BC, H))
    din("W_proj", (H, CIN))
    din("b_proj", (H,))
    din("g_ln", (H2,))
    din("b_ln", (H2,))
    din("g_cn", (H,))
    din("b_cn", (H,))
    din("g_hn", (H,))
    din("b_hn", (H,))
    for g in GATES:
        din(f"W_{g}", (H, H2))
        din(f"b_{g}", (H,))
        din(f"g_{g}", (H,))
        din(f"beta_{g}", (H,))

    out_h = nc.dram_tensor("out_h", (BC, H), F32, kind="ExternalOutput").ap()
    out_c = nc.dram_tensor("out_c", (BC, H), F32, kind="ExternalOutput").ap()

    with tile.TileContext(nc) as tc, ExitStack() as ctx:
        build_body(ctx, tc, ins, out_h, out_c)
    nc.compile()
    return nc


def build_body(ctx, tc, ins, out_h, out_c):
    nc = tc.nc

    # ---------------- deep pools (live through gates and tail) ------------
    singles = ctx.enter_context(tc.tile_pool(name="singles", bufs=1))
    smallp = ctx.enter_context(tc.tile_pool(name="smallp", bufs=1))
    tscr = ctx.enter_context(tc.tile_pool(name="tscr", bufs=1))
    cpool = ctx.enter_context(tc.tile_pool(name="cpool", bufs=1))
    bnp = ctx.enter_context(tc.tile_pool(name="bnp", bufs=1))
    dram = ctx.enter_context(tc.tile_pool(name="dram", bufs=1, space="DRAM"))

    combp = tc.alloc_tile_pool(name="comb", bufs=1)
    comb = [combp.tile([128, BC], BF16, name=f"comb{k}", tag=f"comb{k}")
            for k in range(KC)]

    ident = singles.tile([128, 128], F32)
    make_identity(nc, ident)
    ones_bf = singles.tile([128, 1], BF16)
    nc.vector.memset(ones_bf, 1.0)
    eps_col = singles.tile([128, 1], F32)
    nc.vector.memset(eps_col, 1e-5)
    eps_row = singles.tile([1, 1], F32)
    nc.vector.memset(eps_row, 1e-5)

    cols_req = []

    def load_cols(name, n):
        # Placeholder tile; filled in prep via a contiguous load + PE
        # transpose (a strided [p, c] DMA would head-of-line block the ring).
        t = singles.tile([128, n], F32, name=f"cols_{name}")
        cols_req.append((name, n, t))
        return t

    g_ln = load_cols("g_ln", KC)
    b_ln = load_cols("b_ln", KC)
    b_proj = load_cols("b_proj", FC)

    # ---- weight casts fp32 -> bf16, DRAM -> DRAM on the SWDGE path -------
    # Only gate c2's weights are cast upfront; each later gate's casts are
    # emitted during the previous gate so the gpsimd DMA queue stays short
    # for the per-gate bias/gamma/beta row loads.
    wbf = {g: dram.tile([H, H2], BF16, name=f"wbf_{g}") for g in GATES}

    def emit_wcast(g, triggers):
        """Cast W_g to bf16 in DRAM, chunked per sweep.  The Tile scheduler
        is dependency-driven (emission order alone cannot delay an
        instruction), so each chunk is gated behind a trigger tile via a
        tiny overlapping write: tiny waits for the trigger's producer, the
        big cast overlaps the tiny's destination (WAW) and thus follows it.
        Without this the casts all start at t=0 and saturate the 16 SDMA
        engines exactly when the small prep loads need them."""
        n = len(triggers) if len(triggers) > 2 else SW
        rows = H // n
        for s in range(n):
            trig = triggers[s % len(triggers)]
            nc.gpsimd.dma_start(out=wbf[g][s * rows:s * rows + 1, 0:1],
                                in_=trig[0:1, 0:1])
            nc.gpsimd.dma_start(out=wbf[g][bass.ts(s, rows), :],
                                in_=ins[f"W_{g}"][bass.ts(s, rows), :])

    # ---------------- prep: x^T, h^T, W_proj^T, proj, concat-LN -----------
    prep = tc.alloc_tile_pool(name="prep", bufs=1)
    ppsum = tc.alloc_tile_pool(name="ppsum", bufs=1, space="PSUM")

    def transpose_chunk(src_ap, dst_ap):
        pt = ppsum.tile([128, 128], F32, tag="tp", bufs=2)
        nc.tensor.transpose(pt, src_ap, ident)
        nc.vector.tensor_copy(out=dst_ap, in_=pt)

    # per-partition constant columns: contiguous [n, 128] load + PE transpose
    for name, n, t in cols_req:
        raw = prep.tile([n, 128], F32, name=f"raw_{name}", tag="colraw",
                        bufs=3)
        nc.scalar.dma_start(out=raw,
                            in_=ins[name].rearrange("(c p) -> c p", p=128))
        pt = ppsum.tile([128, KC], F32, tag="cpt", bufs=1)
        nc.tensor.transpose(pt[:, :n], raw, ident[:n, :n])
        nc.vector.tensor_copy(out=t, in_=pt[:, :n])

    # Stage loads split across both HWDGE rings (issue rate is the prep
    # bottleneck): h rows (1MB each) on sync, x/W_proj/cols on scalar.
    xT = [prep.tile([128, BC], BF16, name=f"xT{j}", tag=f"xT{j}")
          for j in range(PC)]
    x2d = ins["x"].rearrange("b one k -> (b one) k")
    trig_hs = None
    for bt in range(NB):
        hs = prep.tile([128, H], F32, tag="hstage", bufs=3)
        nc.sync.dma_start(out=hs, in_=ins["h"][bass.ts(bt, 128), :])
        if bt == 1:
            trig_hs = hs
        xs = prep.tile([128, CIN], F32, tag="xstage", bufs=3)
        nc.scalar.dma_start(out=xs, in_=x2d[bass.ts(bt, 128), :])
        for j in range(PC):
            transpose_chunk(xs[:, bass.ts(j, 128)], xT[j][:, bass.ts(bt, 128)])
        for j in range(FC):
            transpose_chunk(hs[:, bass.ts(j, 128)],
                            comb[FC + j][:, bass.ts(bt, 128)])

    wpT = [prep.tile([128, H], BF16, name=f"wpT{j}", tag=f"wpT{j}")
           for j in range(PC)]
    trig_ws = None
    for f in range(FC):
        ws = prep.tile([128, CIN], F32, tag="wpstage", bufs=4)
        nc.scalar.dma_start(out=ws, in_=ins["W_proj"][bass.ts(f, 128), :])
        if f == 3:
            trig_ws = ws
        for j in range(PC):
            transpose_chunk(ws[:, bass.ts(j, 128)], wpT[j][:, bass.ts(f, 128)])

    # Gate c2's and i's weight casts start once early prep stage loads have
    # landed — they finish before their gates' XBAR streams need them, and
    # gates f/o's casts are triggered off the c2/i weight streams so only
    # a modest cast tail overlaps the XBAR traffic.
    emit_wcast("c2", [trig_hs, trig_ws])
    emit_wcast("i", [trig_ws, trig_hs])

    # xp^T = W_proj @ x^T + b_proj, feature-major into comb[0..FC)
    for f in range(FC):
        pj = [ppsum.tile([128, 512], F32, name=f"pj{f}_{hb}",
                         tag=f"pj{f % 2}_{hb}", bufs=1)
              for hb in range(NHB)]
        for j in range(PC):
            for hb in range(NHB):
                nc.tensor.matmul(pj[hb], wpT[j][:, bass.ts(f, 128)],
                                 xT[j][:, bass.ts(hb, 512)],
                                 start=(j == 0), stop=(j == PC - 1))
        for hb in range(NHB):
            nc.vector.tensor_scalar_add(out=comb[f][:, bass.ts(hb, 512)],
                                        in0=pj[hb], scalar1=b_proj[:, f:f + 1])

    # concat-LN stats: per-batch sum(z), sum(z^2) via ones-matmuls into one
    # PSUM bank (quadrant rows 0/32/64/96).
    ROFF = (0, 32, 64, 96)
    cstat = ppsum.tile([128, 512], F32, tag="stats")
    for k in range(KC):
        for hb in range(NHB):
            zs = comb[k][:, bass.ts(hb, 512)]
            sq = prep.tile([128, 512], BF16, tag="sq", bufs=2)
            nc.scalar.square(sq, zs)
            r0, r1 = ROFF[2 * hb], ROFF[2 * hb + 1]
            nc.tensor.matmul(cstat[r0:r0 + 1, :], ones_bf, zs,
                             start=(k == 0), stop=(k == KC - 1),
                             tile_position=(0, r0))
            nc.tensor.matmul(cstat[r1:r1 + 1, :], ones_bf, sq,
                             start=(k == 0), stop=(k == KC - 1),
                             tile_position=(0, r1))

    m = prep.tile([1, BC], F32, tag="mrow")
    v = prep.tile([1, BC], F32, tag="vrow")
    msq = prep.tile([1, BC], F32, tag="msqrow")
    for hb in range(NHB):
        s = bass.ts(hb, 512)
        r0, r1 = ROFF[2 * hb], ROFF[2 * hb + 1]
        nc.vector.tensor_scalar_mul(m[:, s], cstat[r0:r0 + 1, :], 1.0 / H2)
        nc.vector.tensor_scalar_mul(v[:, s], cstat[r1:r1 + 1, :], 1.0 / H2)
    nc.vector.tensor_mul(msq, m, m)
    nc.vector.tensor_sub(v, v, msq)                       # var
    nc.scalar.activation(out=v, in_=v, func=AF.Sqrt, bias=eps_row, scale=1.0)
    nc.vector.reciprocal(out=v, in_=v)                    # rstd
    nc.vector.tensor_mul(msq, m, v)
    nc.vector.tensor_scalar_mul(msq, msq, -1.0)           # -mean*rstd
    # Broadcast across partitions via a DRAM roundtrip on the HWDGE rings
    # (gpsimd's Q7 is mid cast-descriptor issuance here, so a
    # partition_broadcast would land late — measured slower).
    a_bc = prep.tile([128, BC], F32, tag="abc")
    c_bc = prep.tile([128, BC], F32, tag="cbc")
    for row, bc in ((v, a_bc), (msq, c_bc)):
        drow = dram.tile([1, BC], F32, name="drow", tag="drow", bufs=4)
        nc.sync.dma_start(out=drow, in_=row)
        nc.sync.dma_start(out=bc, in_=_bcast_row(drow))
    for k in range(KC):
        t = prep.tile([128, BC], F32, tag="apply", bufs=4)
        nc.vector.tensor_mul(t, comb[k], a_bc)
        nc.vector.tensor_add(t, t, c_bc)
        nc.scalar.activation(out=comb[k], in_=t, func=AF.Tanh,
                             scale=g_ln[:, k:k + 1], bias=b_ln[:, k:k + 1])

    ppsum.release()
    prep.release()

    # ---------------- gates: batch-major z = comb^T @ W^T ------------------
    zpool = tc.alloc_tile_pool(name="zpool", bufs=1)
    wtp = tc.alloc_tile_pool(name="wtp", bufs=1)
    vbc = tc.alloc_tile_pool(name="vbc", bufs=1)
    gpsum = tc.alloc_tile_pool(name="gpsum", bufs=1, space="PSUM")

    def bcast_vec(pool, name, tag):
        """[H] DRAM fp32 row -> [128, H] bf16 partition-broadcast tile."""
        row = pool.tile([1, H], BF16, name=f"row_{name}", tag="vrow", bufs=1)
        nc.gpsimd.dma_start(out=row, in_=_row(ins[name]))  # cast f32->bf16
        full = pool.tile([128, H], BF16, name=f"bc_{name}", tag=tag, bufs=1)
        nc.gpsimd.partition_broadcast(full, row)
        return full

    def bm_norm_cols(bn_t):
        """bn groups -> (rstd, -mean*rstd) per-partition columns."""
        mv = smallp.tile([128, 2], F32, tag="mv", bufs=8)
        nc.vector.bn_aggr(mv, bn_t)
        sd = smallp.tile([128, 1], F32, tag="sd", bufs=8)
        nc.scalar.activation(out=sd, in_=mv[:, 1:2], func=AF.Sqrt,
                             bias=eps_col, scale=1.0)
        rstd = smallp.tile([128, 1], F32, tag="rstd", bufs=8)
        nc.vector.reciprocal(rstd, sd)
        negm = smallp.tile([128, 1], F32, tag="negm", bufs=8)
        nc.vector.tensor_scalar(out=negm, in0=mv[:, 0:1], scalar1=rstd,
                                scalar2=-1.0, op0=ALU.mult, op1=ALU.mult)
        return rstd, negm

    def bm_apply_slice(dst_ap, src_ap, rstd, negm, g_bc_s, b_bc_s, func):
        """dst = func(((src - m)*rstd)*g + b) on one [128, FS] slice."""
        t = tscr.tile([128, FS], BF16, tag="t", bufs=4)
        nc.scalar.activation(out=t, in_=src_ap, func=AF.Identity,
                             scale=rstd, bias=negm)
        nc.vector.tensor_mul(t, t, g_bc_s)
        nc.vector.tensor_add(t, t, b_bc_s)
        nc.scalar.activation(out=dst_ap, in_=t, func=func)

    zt = {}
    cp = [None] * NB
    bg_work = []        # deferred DVE/ACT closures, interleaved into sweeps
    for g in GATES:
        bb = bcast_vec(vbc, f"b_{g}", "bb")
        gg = bcast_vec(vbc, f"g_{g}", "gg")
        tb = bcast_vec(vbc, f"beta_{g}", "tb")

        z = [zpool.tile([128, H], BF16, name=f"z_{g}{b}", tag=f"{ZTAG[g]}{b}")
             for b in range(NB)]
        bn = [bnp.tile([128, 6 * SW], F32, name=f"bn_{g}{b}", tag=f"bn{b}",
                       bufs=2)
              for b in range(NB)]

        wt_sweep = []
        for s in range(SW):
            ps = [gpsum.tile([128, FS], F32, name=f"ps_{g}{s}_{b}",
                             tag=f"mm{b}", bufs=1)
                  for b in range(NB)]
            for k in range(KC):
                wt = wtp.tile([128, FS], BF16, tag="wt", bufs=12)
                nc.sync.dma_start_transpose(
                    wt, wbf[g][bass.ts(s, FS), bass.ts(k, 128)])
                if k in (0, KC // 2):
                    wt_sweep.append(wt)
                for b in range(NB):
                    nc.tensor.matmul(ps[b], comb[k][:, bass.ts(b, 128)], wt,
                                     start=(k == 0), stop=(k == KC - 1))
            # All 8 PSUM drains first (they gate the next sweep's matmuls),
            # then the stats, which only read SBUF.
            for b in range(NB):
                zs = z[b][:, bass.ts(s, FS)]
                nc.vector.tensor_add(zs, ps[b], bb[:, bass.ts(s, FS)])
            for b in range(NB):
                nc.vector.bn_stats(out=bn[b][:, 6 * s:6 * (s + 1)],
                                   in_=z[b][:, bass.ts(s, FS)])
            for _ in range(2):
                if bg_work:
                    bg_work.pop(0)()

        # Gate g+2's cast chunk s unblocks once this gate's sweep-s weight
        # stream is underway — casts run one full gate ahead of their
        # consumers, spread across sweeps.
        if g == "c2":
            emit_wcast("f", wt_sweep)
        elif g == "i":
            emit_wcast("o", wt_sweep)

        for b in range(NB):
            rstd, negm = bm_norm_cols(bn[b])
            for s in range(SW):
                sl = bass.ts(s, FS)
                bm_apply_slice(z[b][:, sl], z[b][:, sl], rstd, negm,
                               gg[:, sl], tb[:, sl], GATE_FUNC[g])

        zt[g] = z

        if g == "i":
            # cp = i * cc  (cc = gate c2 output, still resident)
            for b in range(NB):
                cp[b] = zpool.tile([128, H], BF16, name=f"cp{b}",
                                   tag=f"cp{b}")
                nc.vector.tensor_mul(cp[b], zt["i"][b], zt["c2"][b])
        elif g == "f":
            # cp += f * c, with c loaded batch-major (cast to bf16 in DMA);
            # then prefetch gate o's weight casts.
            for b in range(NB):
                ct = cpool.tile([128, H], BF16, tag="c", bufs=1)
                nc.gpsimd.dma_start(out=ct, in_=ins["c"][bass.ts(b, 128), :])
                for s in range(SW):
                    sl = bass.ts(s, FS)
                    t = tscr.tile([128, FS], BF16, tag="t", bufs=4)
                    nc.vector.tensor_mul(t, zt["f"][b][:, sl], ct[:, sl])
                    nc.vector.tensor_add(cp[b][:, sl], cp[b][:, sl], t)

            # Cell path: LN_cn(cp) -> out_c, then cp <- tanh(next_cell) in
            # place.  Deferred as closures so the work interleaves into gate
            # o's sweep loop: pure DVE/ACT/DMA that executes while gate o's
            # matmuls occupy the PE.  out_c is written through a bf16
            # cast-DMA (SWDGE) to avoid fp32 staging during the gate window.
            g_cn = bcast_vec(vbc, "g_cn", "g_cn")
            b_cn = bcast_vec(vbc, "b_cn", "b_cn")

            def cell_work(b):
                bn_c = bnp.tile([128, 6 * SW], F32, name=f"bnc{b}",
                                tag=f"bn{b}", bufs=2)
                for s in range(SW):
                    nc.vector.bn_stats(out=bn_c[:, 6 * s:6 * (s + 1)],
                                       in_=cp[b][:, bass.ts(s, FS)])
                rstd, negm = bm_norm_cols(bn_c)
                for s in range(SW):
                    sl = bass.ts(s, FS)
                    t = tscr.tile([128, FS], BF16, tag="t", bufs=4)
                    nc.scalar.activation(out=t, in_=cp[b][:, sl],
                                         func=AF.Identity,
                                         scale=rstd, bias=negm)
                    nc.vector.tensor_mul(t, t, g_cn[:, sl])
                    nc.vector.tensor_add(t, t, b_cn[:, sl])
                    nc.gpsimd.dma_start(out=out_c[bass.ts(b, 128), sl],
                                        in_=t)  # bf16 -> fp32 cast store
                    nc.scalar.activation(out=cp[b][:, sl], in_=t,
                                         func=AF.Tanh)

            bg_work.extend(
                (lambda b=b: cell_work(b)) for b in range(NB))

    while bg_work:
        bg_work.pop(0)()

    gpsum.release()
    vbc.release()
    wtp.release()

    # ---------------- tail: cell LN, hidden path, outputs ------------------
    tailp = tc.alloc_tile_pool(name="tailp", bufs=1)

    def bcast_tail(name):
        row = tailp.tile([1, H], BF16, name=f"row_{name}", tag="trow", bufs=1)
        nc.gpsimd.dma_start(out=row, in_=_row(ins[name]))
        full = tailp.tile([128, H], BF16, name=f"bc_{name}", tag=name, bufs=1)
        nc.gpsimd.partition_broadcast(full, row)
        return full

    g_hn = bcast_tail("g_hn")
    b_hn = bcast_tail("b_hn")

    for b in range(NB):
        # hidden: hp = o * tanh(next_cell) (cp holds the tanh), LN_hn + tanh
        hp = zt["o"][b]
        nc.vector.tensor_mul(hp, hp, cp[b])
        bn_h = bnp.tile([128, 6 * SW], F32, tag=f"bn{b}", bufs=2)
        for s in range(SW):
            nc.vector.bn_stats(out=bn_h[:, 6 * s:6 * (s + 1)],
                               in_=hp[:, bass.ts(s, FS)])
        rstd, negm = bm_norm_cols(bn_h)
        t = tailp.tile([128, H], BF16, tag="ttail", bufs=2)
        nc.scalar.activation(out=t, in_=hp, func=AF.Identity,
                             scale=rstd, bias=negm)
        nc.vector.tensor_mul(t, t, g_hn)
        nc.vector.tensor_add(t, t, b_hn)
        t2 = tailp.tile([128, H], BF16, tag="ttail2", bufs=2)
        nc.scalar.activation(out=t2, in_=t, func=AF.Tanh)
        nc.gpsimd.dma_start(out=out_h[bass.ts(b, 128), :], in_=t2)

    tailp.release()
    zpool.release()
    combp.release()


_NC_CACHE = {}


def _get_nc():
    if "nc" not in _NC_CACHE:
        nc = bacc.Bacc(
            "TRN2",
            target_bir_lowering=False,
            debug=False,
            enable_asserts=False,
            num_devices=NCORES,
        )
        _NC_CACHE["nc"] = build_kernel(nc)
    return _NC_CACHE["nc"]


def run(inputs, **kw):
    nc = _get_nc()
    full = {k: np.ascontiguousarray(np.asarray(v, dtype=np.float32))
            for k, v in inputs.items()}
    in_maps = []
    for i in range(NCORES):
        s = slice(i * BC, (i + 1) * BC)
        m = {k: (np.ascontiguousarray(v[s]) if k in ("x", "h", "c") else v)
             for k, v in full.items()}
        in_maps.append(m)
    res = run_bass_kernel_spmd(nc, in_maps, core_ids=list(range(NCORES)), **kw)
    nh = np.concatenate([r["out_h"] for r in res.results], axis=0)
    ncl = np.concatenate([r["out_c"] for r in res.results], axis=0)
    return np.stack([nh, ncl]).astype(np.float32), res


def kernel(**inputs) -> np.ndarray:
    out, _ = run(inputs)
    return out
